# revision 32
# baseline (speedup 1.0000x reference)
import sys, os
sys.path.insert(0, "/opt/trn_rl_repo")
os.environ.setdefault("NEURON_RT_LOG_LEVEL", "WARNING")
import numpy as np
import ml_dtypes

import concourse.bass as bass
import concourse.bacc as bacc
import concourse.mybir as mybir
import concourse.tile as tile
from concourse import masks
from concourse.bass_utils import run_bass_kernel_spmd

dt = mybir.dt
bf16 = ml_dtypes.bfloat16
NC = 8

N = 50000
NPC_REAL = 6250
TPC = 50
NPAD = TPC * 128          # 6400 positions per core
TA = 25                   # tiles in shard A (half0)
ROWS_A = TA * 128         # 3200 per core
ROWS_B = (TPC - TA) * 128 # 3072 per core
NA = NC * ROWS_A          # 25600 global A rows
NB = NC * ROWS_B          # 24576 global B rows


def balance_positions(indeg0, indeg1, node_ids, nbins, binsize,
                      t0_vec=None, t1_vec=None):
    """2D greedy: assign items into nbins bins of binsize each, filling each
    bin's (sum d0, sum d1) toward per-bin targets (t0_vec, t1_vec).
    Returns array of bin index per item (aligned with node_ids)."""
    d0 = indeg0[node_ids].astype(np.int64)
    d1 = indeg1[node_ids].astype(np.int64)
    order = np.argsort(-(d0 + d1), kind="stable")
    sums0 = np.zeros(nbins, np.int64)
    sums1 = np.zeros(nbins, np.int64)
    fill = np.zeros(nbins, np.int64)
    out = np.zeros(len(node_ids), np.int32)
    t0 = np.full(nbins, max(d0.sum() / nbins, 1.0)) if t0_vec is None \
        else np.maximum(t0_vec.astype(np.float64), 1.0)
    t1 = np.full(nbins, max(d1.sum() / nbins, 1.0)) if t1_vec is None \
        else np.maximum(t1_vec.astype(np.float64), 1.0)
    open_mask = np.ones(nbins, np.bool_)
    for i in order:
        score = np.maximum((sums0 + d0[i]) / t0, (sums1 + d1[i]) / t1)
        score[~open_mask] = np.inf
        b = int(np.argmin(score))
        out[i] = b
        fill[b] += 1
        sums0[b] += d0[i]
        sums1[b] += d1[i]
        if fill[b] >= binsize:
            open_mask[b] = False
    return out


def build_host_data(x, edge_index):
    src_o = edge_index[0].astype(np.int64)
    dst_o = edge_index[1].astype(np.int64)
    indeg = np.bincount(dst_o, minlength=N).astype(np.int64)

    # ---- position assignment (relabeling) ----
    # Half A positions: per core tiles 0..24 (3200), half B: tiles 25..48.
    # Phase 1: split nodes into A-set (25600 slots) and B-set (24576 slots)
    # by total degree snake so halves have similar degree mass; pads fill B.
    nslots = NC * NPAD
    # Assign nodes to half A/B balancing OUT-degree mass ~50/50 (the h-split
    # of edges is by src half; equal halves keep per-half mean segment size
    # under the 1024 rounding boundary) subject to |A| <= NA, |B| <= NB.
    outdeg = np.bincount(src_o, minlength=N).astype(np.int64)
    order = np.argsort(-outdeg, kind="stable")
    setA = np.zeros(N, np.bool_)
    cntA = cntB = 0
    massA = massB = 0
    for n in order:
        d = int(outdeg[n])
        if (massA <= massB and cntA < NA) or cntB >= NB:
            setA[n] = True
            cntA += 1
            massA += d
        else:
            cntB += 1
            massB += d
    nodesA = np.where(setA)[0]
    nodesB = np.where(~setA)[0]

    # indeg by src half requires src half labels = setA (A = half0)
    srcA = setA[src_o]
    indeg0 = np.bincount(dst_o[srcA], minlength=N).astype(np.int64)
    indeg1 = indeg - indeg0

    # Phase 2: 2D balance within each half across (core, tile) bins.
    # Two-tier per-bin capacity targets: most bins target just under 1024
    # (the 128-roundup boundary at chunk granularity 8x128); a few bins per
    # component take the overflow at 896 so total capacity hugs the real
    # edge mass and per-(tile,h) padding collapses.
    def tier_targets(nodes, nb):
        m0 = indeg0[nodes].sum() / NC   # per-core component mass, this half
        m1 = indeg1[nodes].sum() / NC
        nr = nb // NC                   # tile ranks in this half
        marg = 40.0
        k0 = int(np.clip((nr * 1024 - m0 - marg) // 128, 0, nr - 1))
        k1 = int(np.clip((nr * 1024 - m1 - marg) // 128, 0, nr - 1))
        t0 = np.full(nb, 1018.0)
        t1 = np.full(nb, 1018.0)
        t0[: k0 * NC] = 890.0           # low-tier bins (whole ranks)
        t1[nb - k1 * NC:] = 890.0       # disjoint end so tiers spread
        return t0, t1

    t0A, t1A = tier_targets(nodesA, NC * TA)
    t0B, t1B = tier_targets(nodesB, NC * (TPC - TA))
    binsA = balance_positions(indeg0, indeg1, nodesA, NC * TA, 128, t0A, t1A)
    binsB = balance_positions(indeg0, indeg1, nodesB, NC * (TPC - TA), 128,
                              t0B, t1B)

    def bin_slot_map(bins, node_ids, nbins):
        """Assign bins to (core, tile-rank) slots so that the 8 bins sharing a
        tile-rank have similar (sum d0, sum d1) -> max-over-cores ~ mean."""
        s0 = np.zeros(nbins, np.int64)
        s1 = np.zeros(nbins, np.int64)
        np.add.at(s0, bins, indeg0[node_ids])
        np.add.at(s1, bins, indeg1[node_ids])
        rank = np.lexsort((s1, s0 // 16))
        slot_of_bin = np.zeros(nbins, np.int64)
        slot_of_bin[rank] = np.arange(nbins)   # slot r: c=r%NC, trank=r//NC
        return slot_of_bin

    slotA = bin_slot_map(binsA, nodesA, NC * TA)
    slotB = bin_slot_map(binsB, nodesB, NC * (TPC - TA))

    # build position map: pos = core*NPAD + tile*128 + lane
    pos_of = np.full(N, -1, np.int64)
    fillA = np.zeros(NC * TA, np.int64)
    for i, n in enumerate(nodesA):
        b = binsA[i]
        r = slotA[b]
        c, t = r % NC, r // NC
        pos_of[n] = c * NPAD + t * 128 + fillA[b]
        fillA[b] += 1
    fillB = np.zeros(NC * (TPC - TA), np.int64)
    for i, n in enumerate(nodesB):
        b = binsB[i]
        r = slotB[b]
        c, t = r % NC, r // NC
        pos_of[n] = c * NPAD + (TA + t) * 128 + fillB[b]
        fillB[b] += 1
    assert (pos_of[np.concatenate([nodesA, nodesB])] >= 0).all()

    # per-position arrays
    deg_pos = np.ones(nslots, np.float32)
    deg_pos[pos_of] = (indeg + 1).astype(np.float32)
    x_pos = np.zeros((nslots, x.shape[1]), np.float32)
    x_pos[pos_of] = x

    # edges in position space
    src_p = pos_of[src_o]
    dst_p = pos_of[dst_o]
    core = dst_p // NPAD
    dstl = dst_p - core * NPAD
    tl = dstl >> 7
    dl128 = (dstl & 127).astype(np.float32)
    s_core = src_p // NPAD
    s_off = src_p - s_core * NPAD
    h = (s_off >= ROWS_A).astype(np.int64)
    # global gather row within shard A or B
    gidx = np.where(h == 0, s_core * ROWS_A + s_off,
                    s_core * ROWS_B + (s_off - ROWS_A))

    # sort edges by (core, tile, h); run bounds via bincount cumsum
    order_e = np.lexsort((h, tl, core))
    s_gidx = gidx[order_e]
    s_dl = dl128[order_e]

    key = (core * TPC + tl) * 2 + h
    cnt = np.bincount(key, minlength=NC * TPC * 2).reshape(NC, TPC, 2)
    m = cnt.max(axis=0)
    m = ((m + 127) // 128 * 128).astype(np.int64)      # [TPC, 2]

    # stream layout: groups of G tiles; within group h=0 segments first
    G = 4
    segs = [[] for _ in range(TPC)]
    groups = []
    pos = 0
    for g0 in range(0, TPC, G):
        tls = list(range(g0, min(g0 + G, TPC)))
        gmeta = {0: [], 1: []}
        for hh in (0, 1):
            for t in tls:
                L = int(m[t, hh])
                if L:
                    gmeta[hh].append((t, pos, L))
                    segs[t].append((hh, pos, L))
                    pos += L
        groups.append(gmeta)
    TOT = pos
    assert TOT % 128 == 0

    # boundaries of (core, tile, h) runs inside the sorted edge list
    bounds = np.zeros(NC * TPC * 2 + 1, np.int64)
    bounds[1:] = np.cumsum(cnt.reshape(-1))

    per_core = []
    for c in range(NC):
        idx_arr = np.zeros(TOT, np.int32)
        dl_arr = np.full(TOT, -1.0, np.float32)
        for t in range(TPC):
            for hh, spos, L in segs[t]:
                k = (c * TPC + t) * 2 + hh
                a, b = bounds[k], bounds[k + 1]
                n = b - a
                assert n <= L
                idx_arr[spos:spos + n] = s_gidx[a:b]
                dl_arr[spos:spos + n] = s_dl[a:b]
        assert idx_arr.max() < 32768
        idx_w = np.tile(idx_arr.astype(np.int16).reshape(TOT // 16, 16).T,
                        (8, 1)).copy()
        dl_w = dl_arr.reshape(TOT // 128, 128).T.astype(bf16).copy()

        degc = deg_pos[c * NPAD:(c + 1) * NPAD]
        deg_pp = degc.reshape(TPC, 128).T.copy()
        deg_row = degc.reshape(1, NPAD).copy()

        xc = x_pos[c * NPAD:(c + 1) * NPAD]
        xtt = xc.reshape(TPC, 128, 3, 128).transpose(3, 0, 2, 1) \
                .reshape(128, TPC * 3 * 128).astype(bf16)

        per_core.append(dict(idx=idx_w, dl=dl_w, deg_pp=deg_pp,
                             deg_row=deg_row, xtt=xtt))

    cfg = dict(TOT=TOT, segs=segs, groups=groups, G=G, pos_of=pos_of)
    return cfg, per_core


def make_weight_inputs(W1, b1, W2, b2, W3, b3, W4, b4, Wl, bl):
    return dict(
        W1=np.asarray(W1, np.float32).astype(bf16),
        W2=np.asarray(W2, np.float32).astype(bf16),
        W3=np.asarray(W3, np.float32).astype(bf16),
        W4=np.asarray(W4, np.float32).astype(bf16),
        Wl=np.asarray(Wl, np.float32).astype(bf16),
        b1=np.asarray(b1, np.float32).reshape(1, -1).astype(bf16),
        b2=np.asarray(b2, np.float32).reshape(1, -1).astype(bf16),
        b3=np.asarray(b3, np.float32).reshape(1, -1).astype(bf16),
        b4=np.asarray(b4, np.float32).reshape(1, -1).astype(bf16),
        bl=np.asarray(bl, np.float32).reshape(1, -1).astype(bf16),
    )


def split_calls(pos, L, maxc):
    out = []
    while L > 0:
        c = min(L, maxc)
        out.append((pos, c))
        pos += c
        L -= c
    return out


def build_program(cfg, maxc128=8192, maxc256=6144, lrelu=True):
    TOT, segs = cfg["TOT"], cfg["segs"]

    nc = bacc.Bacc("TRN2", target_bir_lowering=False, debug=False,
                   num_devices=NC)

    # ---- I/O ----
    xtt_t = nc.dram_tensor("xtt", [128, TPC * 3 * 128], dt.bfloat16,
                           kind="ExternalInput")
    idx_t = nc.dram_tensor("idx", [128, TOT // 16], dt.int16,
                           kind="ExternalInput")
    dl_t = nc.dram_tensor("dl", [128, TOT // 128], dt.bfloat16,
                          kind="ExternalInput")
    degpp_t = nc.dram_tensor("deg_pp", [128, TPC], dt.float32,
                             kind="ExternalInput")
    degrow_t = nc.dram_tensor("deg_row", [1, NPAD], dt.float32,
                              kind="ExternalInput")
    w_t = {k: nc.dram_tensor(k, list(s), dt.bfloat16, kind="ExternalInput")
           for k, s in dict(W1=(384, 128), W2=(128, 384), W3=(384, 256),
                            W4=(256, 384), Wl=(384, 128), b1=(1, 128),
                            b2=(1, 384), b3=(1, 256), b4=(1, 384),
                            bl=(1, 128)).items()}
    out_t = nc.dram_tensor("out", [NPAD, 128], dt.float32,
                           kind="ExternalOutput")

    # ---- internal DRAM: per-pass local shards + shared tables (A/B) ----
    FDIMS = [128, 128, 256, 256]
    agA = [nc.dram_tensor(f"agA{i}", [ROWS_A, F], dt.bfloat16)
           for i, F in enumerate(FDIMS)]
    agB = [nc.dram_tensor(f"agB{i}", [ROWS_B, F], dt.bfloat16)
           for i, F in enumerate(FDIMS)]
    tabA = [nc.dram_tensor(f"tabA{i}", [NA, F], dt.bfloat16,
                           addr_space="Shared") for i, F in enumerate(FDIMS)]
    tabB = [nc.dram_tensor(f"tabB{i}", [NB, F], dt.bfloat16,
                           addr_space="Shared") for i, F in enumerate(FDIMS)]

    f32, bft = dt.float32, dt.bfloat16

    def ag_row(t):
        """(dram tensor idx-fn, row0) for tile t of pass pi shards."""
        if t < TA:
            return 0, t * 128
        return 1, (t - TA) * 128

    with tile.TileContext(nc) as tc:
        with tc.tile_pool(name="const", bufs=1) as cp:
            iota_i = cp.tile([128, 128], dt.int32)
            nc.gpsimd.iota(iota_i[:], pattern=[[1, 128]], base=0,
                           channel_multiplier=0)
            iota_b = cp.tile([128, 128], bft)
            nc.vector.tensor_copy(iota_b[:], iota_i[:])
            ident_b = cp.tile([128, 128], bft)
            masks.make_identity(nc, ident_b[:])
            ones_row = cp.tile([1, 128], bft)
            nc.gpsimd.memset(ones_row[:], 1.0)

            idx_sb = cp.tile([128, TOT // 16], dt.int16)
            nc.sync.dma_start(out=idx_sb[:], in_=idx_t[:, :])
            dl_sb = cp.tile([128, TOT // 128], bft)
            nc.sync.dma_start(out=dl_sb[:], in_=dl_t[:, :])

            def wtiles(name, K, F):
                ts = []
                for k in range(K // 128):
                    w = cp.tile([128, F], bft, tag=f"{name}{k}")
                    nc.sync.dma_start(out=w[:],
                                      in_=w_t[name][k * 128:(k + 1) * 128, :])
                    ts.append(w)
                return ts
            W1sb = wtiles("W1", 384, 128)
            W2sb = wtiles("W2", 128, 384)
            W3sb = wtiles("W3", 384, 256)
            W4sb = wtiles("W4", 256, 384)
            Wlsb = wtiles("Wl", 384, 128)
            brow = {}
            for name, F in [("b1", 128), ("b2", 384), ("b3", 256),
                            ("b4", 384), ("bl", 128)]:
                b = cp.tile([1, F], bft, tag=name)
                nc.sync.dma_start(out=b[:], in_=w_t[name][:, :])
                brow[name] = b

            deg_pp = cp.tile([128, TPC], f32)
            nc.sync.dma_start(out=deg_pp[:], in_=degpp_t[:, :])
            sq_pp = cp.tile([128, TPC], f32)
            nc.scalar.activation(sq_pp[:], deg_pp[:],
                                 mybir.ActivationFunctionType.Sqrt)
            dinv_pp = cp.tile([128, TPC], f32)
            nc.vector.reciprocal(dinv_pp[:], sq_pp[:])
            deginv_pp = cp.tile([128, TPC], f32)
            nc.vector.reciprocal(deginv_pp[:], deg_pp[:])
            deg_row = cp.tile([1, NPAD], f32)
            nc.sync.dma_start(out=deg_row[:], in_=degrow_t[:, :])
            sq_row = cp.tile([1, NPAD], bft)
            nc.scalar.activation(sq_row[:], deg_row[:],
                                 mybir.ActivationFunctionType.Sqrt)

            if lrelu:
                def act_leaky(out_ap, ps_ap, scale_tile, t, tmp_pool):
                    nc.scalar.activation(out_ap, ps_ap,
                                         mybir.ActivationFunctionType.Lrelu,
                                         bias=0.0, scale=scale_tile[:, t:t + 1],
                                         alpha=0.01)
            else:
                s99 = {}
                s001 = {}
                for nm, tl_ in (("dinv", dinv_pp), ("deginv", deginv_pp)):
                    a = cp.tile([128, TPC], f32, tag=f"{nm}99")
                    nc.vector.tensor_scalar_mul(a[:], tl_[:], 0.99)
                    b = cp.tile([128, TPC], f32, tag=f"{nm}001")
                    nc.vector.tensor_scalar_mul(b[:], tl_[:], 0.01)
                    s99[id(tl_)] = a
                    s001[id(tl_)] = b

                def act_leaky(out_ap, ps_ap, scale_tile, t, tmp_pool):
                    r = tmp_pool.tile([128, out_ap.shape[-1]], f32, tag="lrl_r")
                    nc.scalar.activation(r[:], ps_ap,
                                         mybir.ActivationFunctionType.Relu,
                                         bias=0.0,
                                         scale=s99[id(scale_tile)][:, t:t + 1])
                    t1 = tmp_pool.tile([128, out_ap.shape[-1]], f32, tag="lrl_t")
                    nc.vector.tensor_scalar(t1[:], ps_ap,
                                            s001[id(scale_tile)][:, t:t + 1],
                                            None, mybir.AluOpType.mult)
                    nc.vector.tensor_add(out_ap, r[:], t1[:])

            s2nm_sb = cp.tile([128, NPAD], bft)
            s4nm_sb = cp.tile([128, 2 * NPAD], bft)

            def prod_dma(pi, t, src_ap):
                """write tile t rows of pass-pi local shard"""
                w, r0 = ag_row(t)
                tgt = (agA[pi] if w == 0 else agB[pi])
                nc.sync.dma_start(out=tgt[r0:r0 + 128, :], in_=src_ap)

            def ag_one(pi, which):
                src, tgt = (agA, tabA) if which == 0 else (agB, tabB)
                nc.gpsimd.collective_compute(
                    "AllGather", mybir.AluOpType.bypass,
                    replica_groups=[list(range(NC))],
                    ins=[src[pi].ap().opt()], outs=[tgt[pi].ap().opt()])

            def allgathers(pi):
                ag_one(pi, 0)
                ag_one(pi, 1)

            # ---------- phase B: dense1 -> T1 ----------
            with tc.tile_pool(name="xp", bufs=1) as xp, \
                 tc.tile_pool(name="t1p", bufs=4) as t1p, \
                 tc.tile_pool(name="psB", bufs=4, space="PSUM") as psB:
                xtt_sb = xp.tile([128, TPC * 3 * 128], bft)
                nc.sync.dma_start(out=xtt_sb[:], in_=xtt_t[:, :])
                for t in range(TPC):
                    ps = psB.tile([128, 128], f32, tag="ps1")
                    for k in range(3):
                        r0 = (t * 3 + k) * 128
                        nc.tensor.matmul(ps[:], lhsT=xtt_sb[:, r0:r0 + 128],
                                         rhs=W1sb[k][:],
                                         start=(k == 0), stop=(k == 2))
                    T1t = t1p.tile([128, 128], bft, tag="t1")
                    nc.vector.tensor_scalar(T1t[:], ps[:], dinv_pp[:, t:t + 1],
                                            None, mybir.AluOpType.mult)
                    prod_dma(0, t, T1t[:])
            ag_one(0, 0)

            # ---------- generic aggregation pass ----------
            def agg_pass(pi, F, post, binit_bias=None, psum_bufs=6,
                         early=None):
                maxc = maxc128 if F == 128 else maxc256
                with tc.tile_pool(name=f"g{pi}", bufs=3) as gp, \
                     tc.tile_pool(name=f"pp{pi}", bufs=3) as pp, \
                     tc.tile_pool(name=f"sl{pi}", bufs=3) as slp, \
                     tc.tile_pool(name=f"agg{pi}", bufs=psum_bufs,
                                  space="PSUM") as ap_:

                    def emit_gathers(gmeta, hh):
                        """Issue the gather calls for (group, half); return
                        [(g_tile, cpos, clen)] for later consumption."""
                        src_ap = (tabA[pi] if hh == 0 else tabB[pi])
                        spans = gmeta[hh]
                        if not spans:
                            return []
                        calls = []
                        gpos = spans[0][1]
                        gend = spans[-1][1] + spans[-1][2]
                        for cpos, clen in split_calls(gpos, gend - gpos, maxc):
                            nch = clen // 128
                            g = gp.tile([128, nch * F], bft, tag="g",
                                        name="g")
                            g3 = g[:].rearrange("p (c e) -> p c e", e=F)
                            nc.gpsimd.dma_gather(
                                out_ap=g3, in_ap=src_ap[:, :],
                                idxs_ap=idx_sb[:, cpos // 16:
                                               (cpos + clen) // 16],
                                num_idxs=clen, num_idxs_reg=clen,
                                elem_size=F, single_packet=False)
                            calls.append((g, cpos, clen))
                        return calls

                    def do_group(gmeta, pre_h0=None):
                        tiles = sorted({t for hh in (0, 1)
                                        for t, _, _ in gmeta[hh]})
                        pst = {}
                        left = {t: sum(L for _, _, L in segs[t]) // 128
                                for t in tiles}
                        for t in tiles:
                            ps = ap_.tile([128, F], f32, tag="agg", name="agg")
                            pst[t] = ps
                            if binit_bias is not None:
                                nc.tensor.matmul(
                                    ps[:],
                                    lhsT=sq_row[0:1, t * 128:(t + 1) * 128],
                                    rhs=binit_bias[:], start=True, stop=False)
                            sl = slp.tile([128, F], bft, tag="sl", name="sl")
                            w, r0 = ag_row(t)
                            tgt = (agA[pi] if w == 0 else agB[pi])
                            nc.sync.dma_start(out=sl[:, :],
                                              in_=tgt[r0:r0 + 128, :])
                            nc.tensor.matmul(ps[:], lhsT=ident_b[:], rhs=sl[:],
                                             start=(binit_bias is None),
                                             stop=(left[t] == 0))
                        for hh in (0, 1):
                            spans = gmeta[hh]
                            if not spans:
                                continue
                            if hh == 0 and pre_h0 is not None:
                                calls = pre_h0
                            else:
                                calls = emit_gathers(gmeta, hh)
                            for g, cpos, clen in calls:
                                nch = clen // 128
                                P = pp.tile([128, clen], bft, tag="P",
                                            name="P")
                                P3 = P[:].rearrange("p (c d) -> p c d", d=128)
                                nc.vector.tensor_tensor(
                                    P3,
                                    iota_b[:].unsqueeze(1)
                                        .broadcast_to([128, nch, 128]),
                                    dl_sb[:, cpos // 128:(cpos + clen) // 128]
                                        .unsqueeze(2)
                                        .broadcast_to([128, nch, 128]),
                                    mybir.AluOpType.is_equal)
                                for j in range(nch):
                                    epos = cpos + j * 128
                                    t = next(tt for tt, p0, L in spans
                                             if p0 <= epos < p0 + L)
                                    left[t] -= 1
                                    nc.tensor.matmul(
                                        pst[t][:],
                                        lhsT=P[:, j * 128:(j + 1) * 128],
                                        rhs=g[:, j * F:(j + 1) * F],
                                        start=False, stop=(left[t] == 0))
                        for t in tiles:
                            post(t, pst[t])

                    groups = cfg["groups"]
                    # stagger: emit the first TWO groups' h0 gathers up front
                    # so the gpsimd stream has table-A work covering AG-B's
                    # flight; matmul/psum structure stays in group order.
                    pre0 = emit_gathers(groups[0], 0)
                    # AG-B trigger between the two staggered gathers: its
                    # input is usually ready by pass start, so dispatching
                    # one gather earlier starts AG-B ~32us sooner while the
                    # second staggered gather still covers its flight.
                    if early is not None:
                        early()
                    pre1 = emit_gathers(groups[1], 0)
                    do_group(groups[0], pre_h0=pre0)
                    do_group(groups[1], pre_h0=pre1)
                    for gmeta in groups[2:]:
                        do_group(gmeta)

            # ---------- pass C: agg1 -> T2 ----------
            with tc.tile_pool(name="t2p", bufs=4) as t2p:
                def post_c(t, ps):
                    T2t = t2p.tile([128, 128], bft, tag="t2")
                    act_leaky(T2t[:], ps[:], deginv_pp, t, t2p)
                    prod_dma(1, t, T2t[:])
                agg_pass(0, 128, post_c, binit_bias=brow["b1"],
                         early=lambda: ag_one(0, 1))
            ag_one(1, 0)

            # ---------- pass D: agg2 -> (fused dense2 + dense3) -> T3 ----------
            with tc.tile_pool(name="hp", bufs=6) as hp, \
                 tc.tile_pool(name="t3p", bufs=4) as t3p, \
                 tc.tile_pool(name="psD", bufs=1, space="PSUM") as psD, \
                 tc.tile_pool(name="trD", bufs=1, space="PSUM") as trD:
                def post_d1(t, ps):
                    nc.vector.tensor_copy(s2nm_sb[:, t * 128:(t + 1) * 128],
                                          ps[:])
                    trs = trD.tile([128, 128], bft, tag="trs")
                    nc.tensor.matmul(trs[:],
                                     lhsT=s2nm_sb[:, t * 128:(t + 1) * 128],
                                     rhs=ident_b[:], is_transpose=True)
                    s2t = hp.tile([128, 128], bft, tag="s2t")
                    nc.vector.tensor_copy(s2t[:], trs[:])
                    ps2 = psD.tile([128, 384], f32, tag="ps2")
                    nc.tensor.matmul(ps2[:],
                                     lhsT=sq_row[0:1, t * 128:(t + 1) * 128],
                                     rhs=brow["b2"][:], start=True, stop=False)
                    nc.tensor.matmul(ps2[:], lhsT=s2t[:],
                                     rhs=W2sb[0][:], start=False, stop=True)
                    h2 = hp.tile([128, 384], bft, tag="h2")
                    act_leaky(h2[:], ps2[:], dinv_pp, t, hp)
                    trp = trD.tile([128, 384], bft, tag="tr")
                    for k in range(3):
                        nc.tensor.matmul(trp[:, k * 128:(k + 1) * 128],
                                         lhsT=h2[:, k * 128:(k + 1) * 128],
                                         rhs=ident_b[:], is_transpose=True)
                    h2t = hp.tile([128, 384], bft, tag="h2t")
                    nc.vector.tensor_copy(h2t[:], trp[:])
                    ps3 = psD.tile([128, 256], f32, tag="ps3")
                    for k in range(3):
                        nc.tensor.matmul(ps3[:],
                                         lhsT=h2t[:, k * 128:(k + 1) * 128],
                                         rhs=W3sb[k][:], start=(k == 0),
                                         stop=(k == 2))
                    T3t = t3p.tile([128, 256], bft, tag="t3")
                    nc.vector.tensor_scalar(T3t[:], ps3[:], dinv_pp[:, t:t + 1],
                                            None, mybir.AluOpType.mult)
                    prod_dma(2, t, T3t[:])
                agg_pass(1, 128, post_d1, psum_bufs=4,
                         early=lambda: ag_one(1, 1))
            ag_one(2, 0)

            # ---------- pass E: agg3 -> T4 ----------
            with tc.tile_pool(name="t4p", bufs=4) as t4p:
                def post_e(t, ps):
                    T4t = t4p.tile([128, 256], bft, tag="t4")
                    act_leaky(T4t[:], ps[:], deginv_pp, t, t4p)
                    prod_dma(3, t, T4t[:])
                agg_pass(2, 256, post_e, binit_bias=brow["b3"],
                         early=lambda: ag_one(2, 1))
            ag_one(3, 0)

            # ---------- pass F: agg4 -> (fused dense4 + dense5) -> out ----------
            with tc.tile_pool(name="hp4", bufs=6) as hp4, \
                 tc.tile_pool(name="op", bufs=4) as op, \
                 tc.tile_pool(name="psF", bufs=1, space="PSUM") as psF, \
                 tc.tile_pool(name="trF", bufs=1, space="PSUM") as trF:
                def post_f1(t, ps):
                    nc.vector.tensor_copy(s4nm_sb[:, t * 256:(t + 1) * 256],
                                          ps[:])
                    s4t = hp4.tile([128, 256], bft, tag="s4t")
                    for fk in range(2):
                        trs = trF.tile([128, 128], bft, tag="trs4")
                        nc.tensor.matmul(
                            trs[:],
                            lhsT=s4nm_sb[:, t * 256 + fk * 128:
                                         t * 256 + (fk + 1) * 128],
                            rhs=ident_b[:], is_transpose=True)
                        nc.vector.tensor_copy(s4t[:, fk * 128:(fk + 1) * 128],
                                              trs[:])
                    ps4 = psF.tile([128, 384], f32, tag="ps4")
                    nc.tensor.matmul(ps4[:],
                                     lhsT=sq_row[0:1, t * 128:(t + 1) * 128],
                                     rhs=brow["b4"][:], start=True, stop=False)
                    for fk in range(2):
                        nc.tensor.matmul(ps4[:],
                                         lhsT=s4t[:, fk * 128:(fk + 1) * 128],
                                         rhs=W4sb[fk][:], start=False,
                                         stop=(fk == 1))
                    h4 = hp4.tile([128, 384], bft, tag="h4")
                    act_leaky(h4[:], ps4[:], dinv_pp, t, hp4)
                    trp = trF.tile([128, 384], bft, tag="tr4")
                    for k in range(3):
                        nc.tensor.matmul(trp[:, k * 128:(k + 1) * 128],
                                         lhsT=h4[:, k * 128:(k + 1) * 128],
                                         rhs=ident_b[:], is_transpose=True)
                    h4t = hp4.tile([128, 384], bft, tag="h4t")
                    nc.vector.tensor_copy(h4t[:], trp[:])
                    ps5 = psF.tile([128, 128], f32, tag="ps5")
                    nc.tensor.matmul(ps5[:], lhsT=ones_row[:], rhs=brow["bl"][:],
                                     start=True, stop=False)
                    for k in range(3):
                        nc.tensor.matmul(ps5[:],
                                         lhsT=h4t[:, k * 128:(k + 1) * 128],
                                         rhs=Wlsb[k][:], start=False,
                                         stop=(k == 2))
                    ot = op.tile([128, 128], f32, tag="o")
                    nc.scalar.activation(ot[:], ps5[:],
                                         mybir.ActivationFunctionType.Relu)
                    nc.sync.dma_start(out=out_t[t * 128:(t + 1) * 128, :],
                                      in_=ot[:])
                agg_pass(3, 256, post_f1, psum_bufs=4,
                         early=lambda: ag_one(3, 1))

    nc.compile()
    return nc


def kernel(x, edge_index, W1, b1, W2, b2, W3, b3, W4, b4, Wl, bl,
           trace=False):
    x = np.asarray(x, dtype=np.float32)
    edge_index = np.asarray(edge_index)
    cfg, per_core = build_host_data(x, edge_index)
    wshared = make_weight_inputs(W1, b1, W2, b2, W3, b3, W4, b4, Wl, bl)
    nc = build_program(cfg)
    in_maps = []
    for c in range(NC):
        m = {k: per_core[c][k] for k in
             ("xtt", "idx", "dl", "deg_pp", "deg_row")}
        m.update(wshared)
        in_maps.append(m)
    res = run_bass_kernel_spmd(nc, in_maps, core_ids=list(range(NC)),
                               trace=trace)
    out_pos = np.concatenate([res.results[c]["out"] for c in range(NC)],
                             axis=0)
    out = out_pos[cfg["pos_of"]]
    kernel.last_exec_time_ns = res.exec_time_ns
    kernel.last_results = res
    return out


# revision 33
# speedup vs baseline: 1.0026x; 1.0026x over previous
import sys, os
sys.path.insert(0, "/opt/trn_rl_repo")
os.environ.setdefault("NEURON_RT_LOG_LEVEL", "WARNING")
import numpy as np
import ml_dtypes

import concourse.bass as bass
import concourse.bacc as bacc
import concourse.mybir as mybir
import concourse.tile as tile
from concourse import masks
from concourse.bass_utils import run_bass_kernel_spmd

dt = mybir.dt
bf16 = ml_dtypes.bfloat16
NC = 8

N = 50000
NPC_REAL = 6250
TPC = 50
NPAD = TPC * 128          # 6400 positions per core
TA = 25                   # tiles in shard A (half0)
ROWS_A = TA * 128         # 3200 per core
ROWS_B = (TPC - TA) * 128 # 3072 per core
NA = NC * ROWS_A          # 25600 global A rows
NB = NC * ROWS_B          # 24576 global B rows


def balance_positions(indeg0, indeg1, node_ids, nbins, binsize,
                      t0_vec=None, t1_vec=None):
    """2D greedy: assign items into nbins bins of binsize each, filling each
    bin's (sum d0, sum d1) toward per-bin targets (t0_vec, t1_vec).
    Returns array of bin index per item (aligned with node_ids)."""
    d0 = indeg0[node_ids].astype(np.int64)
    d1 = indeg1[node_ids].astype(np.int64)
    order = np.argsort(-(d0 + d1), kind="stable")
    sums0 = np.zeros(nbins, np.int64)
    sums1 = np.zeros(nbins, np.int64)
    fill = np.zeros(nbins, np.int64)
    out = np.zeros(len(node_ids), np.int32)
    t0 = np.full(nbins, max(d0.sum() / nbins, 1.0)) if t0_vec is None \
        else np.maximum(t0_vec.astype(np.float64), 1.0)
    t1 = np.full(nbins, max(d1.sum() / nbins, 1.0)) if t1_vec is None \
        else np.maximum(t1_vec.astype(np.float64), 1.0)
    open_mask = np.ones(nbins, np.bool_)
    for i in order:
        score = np.maximum((sums0 + d0[i]) / t0, (sums1 + d1[i]) / t1)
        score[~open_mask] = np.inf
        b = int(np.argmin(score))
        out[i] = b
        fill[b] += 1
        sums0[b] += d0[i]
        sums1[b] += d1[i]
        if fill[b] >= binsize:
            open_mask[b] = False
    return out


def build_host_data(x, edge_index):
    src_o = edge_index[0].astype(np.int64)
    dst_o = edge_index[1].astype(np.int64)
    indeg = np.bincount(dst_o, minlength=N).astype(np.int64)

    # ---- position assignment (relabeling) ----
    # Half A positions: per core tiles 0..24 (3200), half B: tiles 25..48.
    # Phase 1: split nodes into A-set (25600 slots) and B-set (24576 slots)
    # by total degree snake so halves have similar degree mass; pads fill B.
    nslots = NC * NPAD
    # Assign nodes to half A/B balancing OUT-degree mass ~50/50 (the h-split
    # of edges is by src half; equal halves keep per-half mean segment size
    # under the 1024 rounding boundary) subject to |A| <= NA, |B| <= NB.
    outdeg = np.bincount(src_o, minlength=N).astype(np.int64)
    order = np.argsort(-outdeg, kind="stable")
    setA = np.zeros(N, np.bool_)
    cntA = cntB = 0
    massA = massB = 0
    for n in order:
        d = int(outdeg[n])
        if (massA <= massB and cntA < NA) or cntB >= NB:
            setA[n] = True
            cntA += 1
            massA += d
        else:
            cntB += 1
            massB += d
    nodesA = np.where(setA)[0]
    nodesB = np.where(~setA)[0]

    # indeg by src half requires src half labels = setA (A = half0)
    srcA = setA[src_o]
    indeg0 = np.bincount(dst_o[srcA], minlength=N).astype(np.int64)
    indeg1 = indeg - indeg0

    # Phase 2: 2D balance within each half across (core, tile) bins.
    # Two-tier per-bin capacity targets: most bins target just under 1024
    # (the 128-roundup boundary at chunk granularity 8x128); a few bins per
    # component take the overflow at 896 so total capacity hugs the real
    # edge mass and per-(tile,h) padding collapses.
    def tier_targets(nodes, nb):
        m0 = indeg0[nodes].sum() / NC   # per-core component mass, this half
        m1 = indeg1[nodes].sum() / NC
        nr = nb // NC                   # tile ranks in this half
        marg = 40.0
        k0 = int(np.clip((nr * 1024 - m0 - marg) // 128, 0, nr - 1))
        k1 = int(np.clip((nr * 1024 - m1 - marg) // 128, 0, nr - 1))
        t0 = np.full(nb, 1018.0)
        t1 = np.full(nb, 1018.0)
        t0[: k0 * NC] = 890.0           # low-tier bins (whole ranks)
        t1[nb - k1 * NC:] = 890.0       # disjoint end so tiers spread
        return t0, t1

    t0A, t1A = tier_targets(nodesA, NC * TA)
    t0B, t1B = tier_targets(nodesB, NC * (TPC - TA))
    binsA = balance_positions(indeg0, indeg1, nodesA, NC * TA, 128, t0A, t1A)
    binsB = balance_positions(indeg0, indeg1, nodesB, NC * (TPC - TA), 128,
                              t0B, t1B)

    def bin_slot_map(bins, node_ids, nbins):
        """Assign bins to (core, tile-rank) slots so that the 8 bins sharing a
        tile-rank have similar (sum d0, sum d1) -> max-over-cores ~ mean."""
        s0 = np.zeros(nbins, np.int64)
        s1 = np.zeros(nbins, np.int64)
        np.add.at(s0, bins, indeg0[node_ids])
        np.add.at(s1, bins, indeg1[node_ids])
        rank = np.lexsort((s1, s0 // 16))
        slot_of_bin = np.zeros(nbins, np.int64)
        slot_of_bin[rank] = np.arange(nbins)   # slot r: c=r%NC, trank=r//NC
        return slot_of_bin

    slotA = bin_slot_map(binsA, nodesA, NC * TA)
    slotB = bin_slot_map(binsB, nodesB, NC * (TPC - TA))

    # build position map: pos = core*NPAD + tile*128 + lane
    pos_of = np.full(N, -1, np.int64)
    fillA = np.zeros(NC * TA, np.int64)
    for i, n in enumerate(nodesA):
        b = binsA[i]
        r = slotA[b]
        c, t = r % NC, r // NC
        pos_of[n] = c * NPAD + t * 128 + fillA[b]
        fillA[b] += 1
    fillB = np.zeros(NC * (TPC - TA), np.int64)
    for i, n in enumerate(nodesB):
        b = binsB[i]
        r = slotB[b]
        c, t = r % NC, r // NC
        pos_of[n] = c * NPAD + (TA + t) * 128 + fillB[b]
        fillB[b] += 1
    assert (pos_of[np.concatenate([nodesA, nodesB])] >= 0).all()

    # per-position arrays
    deg_pos = np.ones(nslots, np.float32)
    deg_pos[pos_of] = (indeg + 1).astype(np.float32)
    x_pos = np.zeros((nslots, x.shape[1]), np.float32)
    x_pos[pos_of] = x

    # edges in position space
    src_p = pos_of[src_o]
    dst_p = pos_of[dst_o]
    core = dst_p // NPAD
    dstl = dst_p - core * NPAD
    tl = dstl >> 7
    dl128 = (dstl & 127).astype(np.float32)
    s_core = src_p // NPAD
    s_off = src_p - s_core * NPAD
    h = (s_off >= ROWS_A).astype(np.int64)
    # global gather row within shard A or B
    gidx = np.where(h == 0, s_core * ROWS_A + s_off,
                    s_core * ROWS_B + (s_off - ROWS_A))

    # sort edges by (core, tile, h); run bounds via bincount cumsum
    order_e = np.lexsort((h, tl, core))
    s_gidx = gidx[order_e]
    s_dl = dl128[order_e]

    key = (core * TPC + tl) * 2 + h
    cnt = np.bincount(key, minlength=NC * TPC * 2).reshape(NC, TPC, 2)
    m = cnt.max(axis=0)
    m = ((m + 127) // 128 * 128).astype(np.int64)      # [TPC, 2]

    # stream layout: groups of G tiles; within group h=0 segments first
    G = 4
    segs = [[] for _ in range(TPC)]
    groups = []
    pos = 0
    for g0 in range(0, TPC, G):
        tls = list(range(g0, min(g0 + G, TPC)))
        gmeta = {0: [], 1: []}
        for hh in (0, 1):
            for t in tls:
                L = int(m[t, hh])
                if L:
                    gmeta[hh].append((t, pos, L))
                    segs[t].append((hh, pos, L))
                    pos += L
        groups.append(gmeta)
    TOT = pos
    assert TOT % 128 == 0

    # boundaries of (core, tile, h) runs inside the sorted edge list
    bounds = np.zeros(NC * TPC * 2 + 1, np.int64)
    bounds[1:] = np.cumsum(cnt.reshape(-1))

    per_core = []
    for c in range(NC):
        idx_arr = np.zeros(TOT, np.int32)
        dl_arr = np.full(TOT, -1.0, np.float32)
        for t in range(TPC):
            for hh, spos, L in segs[t]:
                k = (c * TPC + t) * 2 + hh
                a, b = bounds[k], bounds[k + 1]
                n = b - a
                assert n <= L
                idx_arr[spos:spos + n] = s_gidx[a:b]
                dl_arr[spos:spos + n] = s_dl[a:b]
        assert idx_arr.max() < 32768
        idx_w = np.tile(idx_arr.astype(np.int16).reshape(TOT // 16, 16).T,
                        (8, 1)).copy()
        dl_w = dl_arr.reshape(TOT // 128, 128).T.astype(bf16).copy()

        degc = deg_pos[c * NPAD:(c + 1) * NPAD]
        deg_pp = degc.reshape(TPC, 128).T.copy()
        deg_row = degc.reshape(1, NPAD).copy()

        xc = x_pos[c * NPAD:(c + 1) * NPAD]
        xtt = xc.reshape(TPC, 128, 3, 128).transpose(3, 0, 2, 1) \
                .reshape(128, TPC * 3 * 128).astype(bf16)

        per_core.append(dict(idx=idx_w, dl=dl_w, deg_pp=deg_pp,
                             deg_row=deg_row, xtt=xtt))

    cfg = dict(TOT=TOT, segs=segs, groups=groups, G=G, pos_of=pos_of)
    return cfg, per_core


def make_weight_inputs(W1, b1, W2, b2, W3, b3, W4, b4, Wl, bl):
    return dict(
        W1=np.asarray(W1, np.float32).astype(bf16),
        W2=np.asarray(W2, np.float32).astype(bf16),
        W3=np.asarray(W3, np.float32).astype(bf16),
        W4=np.asarray(W4, np.float32).astype(bf16),
        Wl=np.asarray(Wl, np.float32).astype(bf16),
        b1=np.asarray(b1, np.float32).reshape(1, -1).astype(bf16),
        b2=np.asarray(b2, np.float32).reshape(1, -1).astype(bf16),
        b3=np.asarray(b3, np.float32).reshape(1, -1).astype(bf16),
        b4=np.asarray(b4, np.float32).reshape(1, -1).astype(bf16),
        bl=np.asarray(bl, np.float32).reshape(1, -1).astype(bf16),
    )


def split_calls(pos, L, maxc):
    out = []
    while L > 0:
        c = min(L, maxc)
        out.append((pos, c))
        pos += c
        L -= c
    return out


def build_program(cfg, maxc128=8192, maxc256=6144, lrelu=True):
    TOT, segs = cfg["TOT"], cfg["segs"]

    nc = bacc.Bacc("TRN2", target_bir_lowering=False, debug=False,
                   num_devices=NC)

    # ---- I/O ----
    xtt_t = nc.dram_tensor("xtt", [128, TPC * 3 * 128], dt.bfloat16,
                           kind="ExternalInput")
    idx_t = nc.dram_tensor("idx", [128, TOT // 16], dt.int16,
                           kind="ExternalInput")
    dl_t = nc.dram_tensor("dl", [128, TOT // 128], dt.bfloat16,
                          kind="ExternalInput")
    degpp_t = nc.dram_tensor("deg_pp", [128, TPC], dt.float32,
                             kind="ExternalInput")
    degrow_t = nc.dram_tensor("deg_row", [1, NPAD], dt.float32,
                              kind="ExternalInput")
    w_t = {k: nc.dram_tensor(k, list(s), dt.bfloat16, kind="ExternalInput")
           for k, s in dict(W1=(384, 128), W2=(128, 384), W3=(384, 256),
                            W4=(256, 384), Wl=(384, 128), b1=(1, 128),
                            b2=(1, 384), b3=(1, 256), b4=(1, 384),
                            bl=(1, 128)).items()}
    out_t = nc.dram_tensor("out", [NPAD, 128], dt.float32,
                           kind="ExternalOutput")

    # ---- internal DRAM: per-pass local shards + shared tables (A/B) ----
    FDIMS = [128, 128, 256, 256]
    agA = [nc.dram_tensor(f"agA{i}", [ROWS_A, F], dt.bfloat16)
           for i, F in enumerate(FDIMS)]
    agB = [nc.dram_tensor(f"agB{i}", [ROWS_B, F], dt.bfloat16)
           for i, F in enumerate(FDIMS)]
    tabA = [nc.dram_tensor(f"tabA{i}", [NA, F], dt.bfloat16,
                           addr_space="Shared") for i, F in enumerate(FDIMS)]
    tabB = [nc.dram_tensor(f"tabB{i}", [NB, F], dt.bfloat16,
                           addr_space="Shared") for i, F in enumerate(FDIMS)]

    f32, bft = dt.float32, dt.bfloat16

    def ag_row(t):
        """(dram tensor idx-fn, row0) for tile t of pass pi shards."""
        if t < TA:
            return 0, t * 128
        return 1, (t - TA) * 128

    with tile.TileContext(nc) as tc:
        with tc.tile_pool(name="const", bufs=1) as cp:
            iota_i = cp.tile([128, 128], dt.int32)
            nc.gpsimd.iota(iota_i[:], pattern=[[1, 128]], base=0,
                           channel_multiplier=0)
            iota_b = cp.tile([128, 128], bft)
            nc.vector.tensor_copy(iota_b[:], iota_i[:])
            ident_b = cp.tile([128, 128], bft)
            masks.make_identity(nc, ident_b[:])
            ones_row = cp.tile([1, 128], bft)
            nc.gpsimd.memset(ones_row[:], 1.0)

            idx_sb = cp.tile([128, TOT // 16], dt.int16)
            nc.sync.dma_start(out=idx_sb[:], in_=idx_t[:, :])
            dl_sb = cp.tile([128, TOT // 128], bft)
            nc.sync.dma_start(out=dl_sb[:], in_=dl_t[:, :])

            def wtiles(name, K, F):
                ts = []
                for k in range(K // 128):
                    w = cp.tile([128, F], bft, tag=f"{name}{k}")
                    nc.sync.dma_start(out=w[:],
                                      in_=w_t[name][k * 128:(k + 1) * 128, :])
                    ts.append(w)
                return ts
            W1sb = wtiles("W1", 384, 128)
            W2sb = wtiles("W2", 128, 384)
            W3sb = wtiles("W3", 384, 256)
            W4sb = wtiles("W4", 256, 384)
            Wlsb = wtiles("Wl", 384, 128)
            brow = {}
            for name, F in [("b1", 128), ("b2", 384), ("b3", 256),
                            ("b4", 384), ("bl", 128)]:
                b = cp.tile([1, F], bft, tag=name)
                nc.sync.dma_start(out=b[:], in_=w_t[name][:, :])
                brow[name] = b

            deg_pp = cp.tile([128, TPC], f32)
            nc.sync.dma_start(out=deg_pp[:], in_=degpp_t[:, :])
            sq_pp = cp.tile([128, TPC], f32)
            nc.scalar.activation(sq_pp[:], deg_pp[:],
                                 mybir.ActivationFunctionType.Sqrt)
            dinv_pp = cp.tile([128, TPC], f32)
            nc.vector.reciprocal(dinv_pp[:], sq_pp[:])
            deginv_pp = cp.tile([128, TPC], f32)
            nc.vector.reciprocal(deginv_pp[:], deg_pp[:])
            deg_row = cp.tile([1, NPAD], f32)
            nc.sync.dma_start(out=deg_row[:], in_=degrow_t[:, :])
            sq_row = cp.tile([1, NPAD], bft)
            nc.scalar.activation(sq_row[:], deg_row[:],
                                 mybir.ActivationFunctionType.Sqrt)

            if lrelu:
                def act_leaky(out_ap, ps_ap, scale_tile, t, tmp_pool):
                    nc.scalar.activation(out_ap, ps_ap,
                                         mybir.ActivationFunctionType.Lrelu,
                                         bias=0.0, scale=scale_tile[:, t:t + 1],
                                         alpha=0.01)
            else:
                s99 = {}
                s001 = {}
                for nm, tl_ in (("dinv", dinv_pp), ("deginv", deginv_pp)):
                    a = cp.tile([128, TPC], f32, tag=f"{nm}99")
                    nc.vector.tensor_scalar_mul(a[:], tl_[:], 0.99)
                    b = cp.tile([128, TPC], f32, tag=f"{nm}001")
                    nc.vector.tensor_scalar_mul(b[:], tl_[:], 0.01)
                    s99[id(tl_)] = a
                    s001[id(tl_)] = b

                def act_leaky(out_ap, ps_ap, scale_tile, t, tmp_pool):
                    r = tmp_pool.tile([128, out_ap.shape[-1]], f32, tag="lrl_r")
                    nc.scalar.activation(r[:], ps_ap,
                                         mybir.ActivationFunctionType.Relu,
                                         bias=0.0,
                                         scale=s99[id(scale_tile)][:, t:t + 1])
                    t1 = tmp_pool.tile([128, out_ap.shape[-1]], f32, tag="lrl_t")
                    nc.vector.tensor_scalar(t1[:], ps_ap,
                                            s001[id(scale_tile)][:, t:t + 1],
                                            None, mybir.AluOpType.mult)
                    nc.vector.tensor_add(out_ap, r[:], t1[:])

            s2nm_sb = cp.tile([128, NPAD], bft)
            s4nm_sb = cp.tile([128, 2 * NPAD], bft)

            def prod_dma(pi, t, src_ap):
                """write tile t rows of pass-pi local shard"""
                w, r0 = ag_row(t)
                tgt = (agA[pi] if w == 0 else agB[pi])
                nc.sync.dma_start(out=tgt[r0:r0 + 128, :], in_=src_ap)

            def ag_one(pi, which):
                src, tgt = (agA, tabA) if which == 0 else (agB, tabB)
                nc.gpsimd.collective_compute(
                    "AllGather", mybir.AluOpType.bypass,
                    replica_groups=[list(range(NC))],
                    ins=[src[pi].ap().opt()], outs=[tgt[pi].ap().opt()])

            def allgathers(pi):
                ag_one(pi, 0)
                ag_one(pi, 1)

            # ---------- phase B: dense1 -> T1 ----------
            with tc.tile_pool(name="xp", bufs=1) as xp, \
                 tc.tile_pool(name="t1p", bufs=4) as t1p, \
                 tc.tile_pool(name="psB", bufs=4, space="PSUM") as psB:
                xtt_sb = xp.tile([128, TPC * 3 * 128], bft)
                nc.sync.dma_start(out=xtt_sb[:], in_=xtt_t[:, :])
                for t in range(TPC):
                    ps = psB.tile([128, 128], f32, tag="ps1")
                    for k in range(3):
                        r0 = (t * 3 + k) * 128
                        nc.tensor.matmul(ps[:], lhsT=xtt_sb[:, r0:r0 + 128],
                                         rhs=W1sb[k][:],
                                         start=(k == 0), stop=(k == 2))
                    T1t = t1p.tile([128, 128], bft, tag="t1")
                    nc.vector.tensor_scalar(T1t[:], ps[:], dinv_pp[:, t:t + 1],
                                            None, mybir.AluOpType.mult)
                    prod_dma(0, t, T1t[:])
            ag_one(0, 0)

            # ---------- generic aggregation pass ----------
            def agg_pass(pi, F, post, binit_bias=None, psum_bufs=6,
                         early=None):
                maxc = maxc128 if F == 128 else maxc256
                with tc.tile_pool(name=f"g{pi}", bufs=3) as gp, \
                     tc.tile_pool(name=f"pp{pi}", bufs=3) as pp, \
                     tc.tile_pool(name=f"sl{pi}", bufs=3) as slp, \
                     tc.tile_pool(name=f"agg{pi}", bufs=psum_bufs,
                                  space="PSUM") as ap_:

                    def emit_gathers(gmeta, hh):
                        """Issue the gather calls for (group, half); return
                        [(g_tile, cpos, clen)] for later consumption."""
                        src_ap = (tabA[pi] if hh == 0 else tabB[pi])
                        spans = gmeta[hh]
                        if not spans:
                            return []
                        calls = []
                        gpos = spans[0][1]
                        gend = spans[-1][1] + spans[-1][2]
                        for cpos, clen in split_calls(gpos, gend - gpos, maxc):
                            nch = clen // 128
                            g = gp.tile([128, nch * F], bft, tag="g",
                                        name="g")
                            g3 = g[:].rearrange("p (c e) -> p c e", e=F)
                            nc.gpsimd.dma_gather(
                                out_ap=g3, in_ap=src_ap[:, :],
                                idxs_ap=idx_sb[:, cpos // 16:
                                               (cpos + clen) // 16],
                                num_idxs=clen, num_idxs_reg=clen,
                                elem_size=F, single_packet=False)
                            calls.append((g, cpos, clen))
                        return calls

                    def do_group(gmeta, pre_h0=None):
                        tiles = sorted({t for hh in (0, 1)
                                        for t, _, _ in gmeta[hh]})
                        pst = {}
                        left = {t: sum(L for _, _, L in segs[t]) // 128
                                for t in tiles}
                        for t in tiles:
                            ps = ap_.tile([128, F], f32, tag="agg", name="agg")
                            pst[t] = ps
                            if binit_bias is not None:
                                nc.tensor.matmul(
                                    ps[:],
                                    lhsT=sq_row[0:1, t * 128:(t + 1) * 128],
                                    rhs=binit_bias[:], start=True, stop=False)
                            sl = slp.tile([128, F], bft, tag="sl", name="sl")
                            w, r0 = ag_row(t)
                            tgt = (agA[pi] if w == 0 else agB[pi])
                            nc.sync.dma_start(out=sl[:, :],
                                              in_=tgt[r0:r0 + 128, :])
                            nc.tensor.matmul(ps[:], lhsT=ident_b[:], rhs=sl[:],
                                             start=(binit_bias is None),
                                             stop=(left[t] == 0))
                        for hh in (0, 1):
                            spans = gmeta[hh]
                            if not spans:
                                continue
                            if hh == 0 and pre_h0 is not None:
                                calls = pre_h0
                            else:
                                calls = emit_gathers(gmeta, hh)
                            for g, cpos, clen in calls:
                                nch = clen // 128
                                P = pp.tile([128, clen], bft, tag="P",
                                            name="P")
                                P3 = P[:].rearrange("p (c d) -> p c d", d=128)
                                nc.vector.tensor_tensor(
                                    P3,
                                    iota_b[:].unsqueeze(1)
                                        .broadcast_to([128, nch, 128]),
                                    dl_sb[:, cpos // 128:(cpos + clen) // 128]
                                        .unsqueeze(2)
                                        .broadcast_to([128, nch, 128]),
                                    mybir.AluOpType.is_equal)
                                for j in range(nch):
                                    epos = cpos + j * 128
                                    t = next(tt for tt, p0, L in spans
                                             if p0 <= epos < p0 + L)
                                    left[t] -= 1
                                    nc.tensor.matmul(
                                        pst[t][:],
                                        lhsT=P[:, j * 128:(j + 1) * 128],
                                        rhs=g[:, j * F:(j + 1) * F],
                                        start=False, stop=(left[t] == 0))
                        for t in tiles:
                            post(t, pst[t])

                    groups = cfg["groups"]
                    # stagger: emit the first TWO groups' h0 gathers up front
                    # so the gpsimd stream has table-A work covering AG-B's
                    # flight; matmul/psum structure stays in group order.
                    pre0 = emit_gathers(groups[0], 0)
                    pre1 = emit_gathers(groups[1], 0)
                    if early is not None:
                        early()
                    do_group(groups[0], pre_h0=pre0)
                    do_group(groups[1], pre_h0=pre1)
                    for gmeta in groups[2:]:
                        do_group(gmeta)

            # ---------- pass C: agg1 -> T2 ----------
            with tc.tile_pool(name="t2p", bufs=4) as t2p:
                def post_c(t, ps):
                    T2t = t2p.tile([128, 128], bft, tag="t2")
                    act_leaky(T2t[:], ps[:], deginv_pp, t, t2p)
                    prod_dma(1, t, T2t[:])
                agg_pass(0, 128, post_c, binit_bias=brow["b1"],
                         early=lambda: ag_one(0, 1))
            ag_one(1, 0)

            # ---------- pass D: agg2 -> (fused dense2 + dense3) -> T3 ----------
            with tc.tile_pool(name="hp", bufs=6) as hp, \
                 tc.tile_pool(name="t3p", bufs=4) as t3p, \
                 tc.tile_pool(name="psD", bufs=1, space="PSUM") as psD, \
                 tc.tile_pool(name="trD", bufs=1, space="PSUM") as trD:
                def post_d1(t, ps):
                    nc.vector.tensor_copy(s2nm_sb[:, t * 128:(t + 1) * 128],
                                          ps[:])
                    trs = trD.tile([128, 128], bft, tag="trs")
                    nc.tensor.matmul(trs[:],
                                     lhsT=s2nm_sb[:, t * 128:(t + 1) * 128],
                                     rhs=ident_b[:], is_transpose=True)
                    s2t = hp.tile([128, 128], bft, tag="s2t")
                    nc.vector.tensor_copy(s2t[:], trs[:])
                    ps2 = psD.tile([128, 384], f32, tag="ps2")
                    nc.tensor.matmul(ps2[:],
                                     lhsT=sq_row[0:1, t * 128:(t + 1) * 128],
                                     rhs=brow["b2"][:], start=True, stop=False)
                    nc.tensor.matmul(ps2[:], lhsT=s2t[:],
                                     rhs=W2sb[0][:], start=False, stop=True)
                    h2 = hp.tile([128, 384], bft, tag="h2")
                    act_leaky(h2[:], ps2[:], dinv_pp, t, hp)
                    trp = trD.tile([128, 384], bft, tag="tr")
                    for k in range(3):
                        nc.tensor.matmul(trp[:, k * 128:(k + 1) * 128],
                                         lhsT=h2[:, k * 128:(k + 1) * 128],
                                         rhs=ident_b[:], is_transpose=True)
                    h2t = hp.tile([128, 384], bft, tag="h2t")
                    nc.vector.tensor_copy(h2t[:], trp[:])
                    ps3 = psD.tile([128, 256], f32, tag="ps3")
                    for k in range(3):
                        nc.tensor.matmul(ps3[:],
                                         lhsT=h2t[:, k * 128:(k + 1) * 128],
                                         rhs=W3sb[k][:], start=(k == 0),
                                         stop=(k == 2))
                    T3t = t3p.tile([128, 256], bft, tag="t3")
                    nc.vector.tensor_scalar(T3t[:], ps3[:], dinv_pp[:, t:t + 1],
                                            None, mybir.AluOpType.mult)
                    prod_dma(2, t, T3t[:])
                agg_pass(1, 128, post_d1, psum_bufs=4,
                         early=lambda: ag_one(1, 1))
            ag_one(2, 0)

            # ---------- pass E: agg3 -> T4 ----------
            with tc.tile_pool(name="t4p", bufs=4) as t4p:
                def post_e(t, ps):
                    T4t = t4p.tile([128, 256], bft, tag="t4")
                    act_leaky(T4t[:], ps[:], deginv_pp, t, t4p)
                    prod_dma(3, t, T4t[:])
                agg_pass(2, 256, post_e, binit_bias=brow["b3"],
                         early=lambda: ag_one(2, 1))
            ag_one(3, 0)

            # ---------- pass F: agg4 -> (fused dense4 + dense5) -> out ----------
            with tc.tile_pool(name="hp4", bufs=6) as hp4, \
                 tc.tile_pool(name="op", bufs=4) as op, \
                 tc.tile_pool(name="psF", bufs=1, space="PSUM") as psF, \
                 tc.tile_pool(name="trF", bufs=1, space="PSUM") as trF:
                def post_f1(t, ps):
                    nc.vector.tensor_copy(s4nm_sb[:, t * 256:(t + 1) * 256],
                                          ps[:])
                    s4t = hp4.tile([128, 256], bft, tag="s4t")
                    for fk in range(2):
                        trs = trF.tile([128, 128], bft, tag="trs4")
                        nc.tensor.matmul(
                            trs[:],
                            lhsT=s4nm_sb[:, t * 256 + fk * 128:
                                         t * 256 + (fk + 1) * 128],
                            rhs=ident_b[:], is_transpose=True)
                        nc.vector.tensor_copy(s4t[:, fk * 128:(fk + 1) * 128],
                                              trs[:])
                    ps4 = psF.tile([128, 384], f32, tag="ps4")
                    nc.tensor.matmul(ps4[:],
                                     lhsT=sq_row[0:1, t * 128:(t + 1) * 128],
                                     rhs=brow["b4"][:], start=True, stop=False)
                    for fk in range(2):
                        nc.tensor.matmul(ps4[:],
                                         lhsT=s4t[:, fk * 128:(fk + 1) * 128],
                                         rhs=W4sb[fk][:], start=False,
                                         stop=(fk == 1))
                    h4 = hp4.tile([128, 384], bft, tag="h4")
                    act_leaky(h4[:], ps4[:], dinv_pp, t, hp4)
                    trp = trF.tile([128, 384], bft, tag="tr4")
                    for k in range(3):
                        nc.tensor.matmul(trp[:, k * 128:(k + 1) * 128],
                                         lhsT=h4[:, k * 128:(k + 1) * 128],
                                         rhs=ident_b[:], is_transpose=True)
                    h4t = hp4.tile([128, 384], bft, tag="h4t")
                    nc.vector.tensor_copy(h4t[:], trp[:])
                    ps5 = psF.tile([128, 128], f32, tag="ps5")
                    nc.tensor.matmul(ps5[:], lhsT=ones_row[:], rhs=brow["bl"][:],
                                     start=True, stop=False)
                    for k in range(3):
                        nc.tensor.matmul(ps5[:],
                                         lhsT=h4t[:, k * 128:(k + 1) * 128],
                                         rhs=Wlsb[k][:], start=False,
                                         stop=(k == 2))
                    ot = op.tile([128, 128], f32, tag="o")
                    nc.scalar.activation(ot[:], ps5[:],
                                         mybir.ActivationFunctionType.Relu)
                    nc.sync.dma_start(out=out_t[t * 128:(t + 1) * 128, :],
                                      in_=ot[:])
                agg_pass(3, 256, post_f1, psum_bufs=4,
                         early=lambda: ag_one(3, 1))

    nc.compile()
    return nc


def kernel(x, edge_index, W1, b1, W2, b2, W3, b3, W4, b4, Wl, bl,
           trace=False):
    x = np.asarray(x, dtype=np.float32)
    edge_index = np.asarray(edge_index)
    cfg, per_core = build_host_data(x, edge_index)
    wshared = make_weight_inputs(W1, b1, W2, b2, W3, b3, W4, b4, Wl, bl)
    nc = build_program(cfg)
    in_maps = []
    for c in range(NC):
        m = {k: per_core[c][k] for k in
             ("xtt", "idx", "dl", "deg_pp", "deg_row")}
        m.update(wshared)
        in_maps.append(m)
    res = run_bass_kernel_spmd(nc, in_maps, core_ids=list(range(NC)),
                               trace=trace)
    out_pos = np.concatenate([res.results[c]["out"] for c in range(NC)],
                             axis=0)
    out = out_pos[cfg["pos_of"]]
    kernel.last_exec_time_ns = res.exec_time_ns
    kernel.last_results = res
    return out
